# revision 1
# baseline (speedup 1.0000x reference)
"""Trainium2 Bass kernel for loss = sum((X[:,None]*A - I)**2), N=8192.

Algebraic decomposition (avoids materializing the residual):
    loss = sum_ij (x_i*a_ij)^2  -  2*sum_i x_i*a_ii  +  N
         = sum_i x_i^2 * r_i    -  2*sum_i x_i*d_i   +  N
where r_i = sum_j a_ij^2 (row sums of squares) and d_i = a_ii.

Sharding: A row-wise across 8 cores (1024 rows each). Each core streams its
32 MB shard from HBM once in [128, 8192] tiles; ScalarE's fused
activation(Square, accum_out) computes per-row sums of squares in a single
pass per tile (~7 us/tile, well under the ~12 us/tile DMA floor, so the
kernel stays memory-bound). A short VectorE epilogue folds in x and the
diagonal, GPSIMD reduces across partitions, and the host sums the 8 scalar
partials (+N) in float64.
"""

import numpy as np

import concourse.bacc as bacc
import concourse.mybir as mybir
from concourse.tile import TileContext
from concourse.bass_utils import run_bass_kernel_spmd

N = 8192
NCORES = 8
ROWS = N // NCORES  # 1024 rows per core
P = 128  # SBUF partitions
TILES = ROWS // P  # 8 row-tiles of 128 rows per core
F = N  # full-row chunk: [128, 8192] f32 = 4 MiB per DMA

_DT = mybir.dt.float32


def build_nc(reps=1):
    """reps>1 repeats the whole per-core computation in one NEFF; used by
    the timing harness to measure per-iteration device time by slope."""
    nc = bacc.Bacc("TRN2", target_bir_lowering=False)

    a_shard = nc.dram_tensor("a_shard", [ROWS, N], _DT, kind="ExternalInput")
    x_shard = nc.dram_tensor("x_shard", [P, TILES], _DT, kind="ExternalInput")
    d_shard = nc.dram_tensor("d_shard", [P, TILES], _DT, kind="ExternalInput")
    out = nc.dram_tensor("out", [P, reps], _DT, kind="ExternalOutput")

    a_tiles = a_shard.rearrange("(t p) n -> t p n", p=P)

    with TileContext(nc) as tc:
        with (
            tc.tile_pool(name="a", bufs=4) as apool,
            tc.tile_pool(name="small", bufs=1) as small,
        ):
            racc = small.tile([P, TILES], _DT, tag="racc")
            xst = small.tile([P, TILES], _DT, tag="xs")
            dst = small.tile([P, TILES], _DT, tag="ds")
            nc.sync.dma_start(out=xst[:], in_=x_shard[:])
            nc.sync.dma_start(out=dst[:], in_=d_shard[:])

            # Throwaway full-size output for the fused square+reduce:
            # stride-0 broadcast of a [P,1] tile, so no [P,F] scratch is
            # needed (qr.py's safe_norm trick).
            dummy = small.tile([P, 1], _DT, tag="dummy")

            for _rep in range(reps):
                for t in range(TILES):
                    at = apool.tile([P, F], _DT, tag="a")
                    nc.sync.dma_start(out=at[:], in_=a_tiles[t])
                    nc.scalar.activation(
                        out=dummy.broadcast_to(at.shape),
                        in_=at[:],
                        func=mybir.ActivationFunctionType.Square,
                        accum_out=racc[:, t : t + 1],
                    )

                # partial = sum_{p,t} x*(r*x - 2*d)
                t1 = small.tile([P, TILES], _DT, tag="t1")
                nc.vector.tensor_mul(out=t1[:], in0=racc[:], in1=xst[:])
                t2 = small.tile([P, TILES], _DT, tag="t2")
                nc.vector.scalar_tensor_tensor(
                    out=t2[:],
                    in0=dst[:],
                    scalar=-2.0,
                    in1=t1[:],
                    op0=mybir.AluOpType.mult,
                    op1=mybir.AluOpType.add,
                )
                t3 = small.tile([P, TILES], _DT, tag="t3")
                nc.vector.tensor_mul(out=t3[:], in0=t2[:], in1=xst[:])
                comb = small.tile([P, 1], _DT, tag="comb")
                nc.vector.reduce_sum(comb[:], t3[:], axis=mybir.AxisListType.X)
                # Ship the [128,1] per-partition partials; the host does the
                # final 1024-value sum in float64 (better precision than a
                # sequential fp32 partition reduce of ~65K-magnitude terms).
                nc.sync.dma_start(out=out[:, _rep : _rep + 1], in_=comb[:])

    nc.compile()
    return nc


_nc_cache = {}


def _get_nc(reps=1):
    if reps not in _nc_cache:
        _nc_cache[reps] = build_nc(reps)
    return _nc_cache[reps]


def _shard_inputs(X, A):
    X = np.ascontiguousarray(np.asarray(X, dtype=np.float32))
    A = np.ascontiguousarray(np.asarray(A, dtype=np.float32))
    d = np.ascontiguousarray(A.diagonal()).astype(np.float32)
    in_maps = []
    for c in range(NCORES):
        r0 = c * ROWS
        in_maps.append(
            {
                "a_shard": A[r0 : r0 + ROWS],
                "x_shard": np.ascontiguousarray(
                    X[r0 : r0 + ROWS].reshape(TILES, P).T
                ),
                "d_shard": np.ascontiguousarray(
                    d[r0 : r0 + ROWS].reshape(TILES, P).T
                ),
            }
        )
    return in_maps


def _run(inputs, trace=False):
    nc = _get_nc()
    in_maps = _shard_inputs(inputs["X"], inputs["A"])
    res = run_bass_kernel_spmd(
        nc, in_maps, core_ids=list(range(NCORES)), trace=trace
    )
    partials = np.array(
        [r["out"][:, 0].astype(np.float64).sum() for r in res.results],
        dtype=np.float64,
    )
    total = np.float32(partials.sum() + float(N))
    return np.array(total, dtype=np.float32), res


def kernel(**inputs):
    out, _ = _run(inputs, trace=False)
    return out



# revision 3
# speedup vs baseline: 1.1023x; 1.1023x over previous
"""Trainium2 Bass kernel for loss = sum((X[:,None]*A - I)**2), N=8192.

Algebraic decomposition (avoids materializing the residual):
    loss = sum_i x_i^2 * r_i  -  2*sum_i x_i*a_ii  +  N
where r_i = sum_j a_ij^2 (row sums of squares of A).

Device work is reduced to pure streaming: each core reads its 32 MiB
row-shard of A once and produces per-row-chunk sums of squares via
ScalarE's fused activation(Square, accum_out). Everything else (x^2
weighting, the diagonal term, partition/core reduction) runs on the
host in float64 from the tiny [128, n_chunks] result.

Perf structure (from the v1 trace: 130 us total vs ~94 us HBM roofline):
  - Dual HWDGE rings: ring A (nc.sync) carries tiles 0,2,4 + a tapered
    tile 6; ring B (nc.scalar) carries tiles 1,3,5 + tapered tile 7.
    Balanced 16 MiB per ring, so the ~1.6 us completion-receipt bubble
    between consecutive DMAs on one ring is absorbed by the other ring
    (the 16 SDMA engines round-robin across queues at packet
    granularity).
  - Tapered tails [4096, 2048, 1536, 512] columns per ring with
    dedicated buffers, so the final ACT after the last byte lands is
    ~0.7 us instead of 8.5 us.
  - No x/d input DMAs and no vector epilogue: the first instruction on
    each ring is already A-data.
"""

import numpy as np

import concourse.bacc as bacc
import concourse.mybir as mybir
from concourse.tile import TileContext
from concourse.bass_utils import run_bass_kernel_spmd

N = 8192
NCORES = 8
ROWS = N // NCORES  # 1024 rows per core
P = 128  # SBUF partitions
TILES = ROWS // P  # 8 row-tiles of 128 rows per core

_DT = mybir.dt.float32

# Per-ring taper of the last row-tile (columns).
TAPER = [4096, 2048, 1536, 512]
assert sum(TAPER) == N

# Chunk schedule: (tile, col0, width, ring). Ring 0 = nc.sync, 1 = nc.scalar.
# Emission order = program order = per-ring queue order.
CHUNKS = []
for t in range(6):
    CHUNKS.append((t, 0, N, t % 2))
_off = 0
for w in TAPER:
    CHUNKS.append((6, _off, w, 0))
    CHUNKS.append((7, _off, w, 1))
    _off += w
NCH = len(CHUNKS)  # 14


def build_nc():
    nc = bacc.Bacc("TRN2", target_bir_lowering=False)

    a_shard = nc.dram_tensor("a_shard", [ROWS, N], _DT, kind="ExternalInput")
    out = nc.dram_tensor("out", [P, NCH], _DT, kind="ExternalOutput")

    a_tiles = a_shard.rearrange("(t p) n -> t p n", p=P)

    with TileContext(nc) as tc:
        with (
            tc.tile_pool(name="big", bufs=4) as big,
            tc.tile_pool(name="t4096", bufs=2) as t4096,
            tc.tile_pool(name="t2048", bufs=2) as t2048,
            tc.tile_pool(name="t1536", bufs=2) as t1536,
            tc.tile_pool(name="t512", bufs=2) as t512,
            tc.tile_pool(name="small", bufs=1) as small,
        ):
            pools = {N: (big, "big"), 4096: (t4096, "t4"), 2048: (t2048, "t2"),
                     1536: (t1536, "t1"), 512: (t512, "t5")}
            racc = small.tile([P, NCH], _DT, tag="racc")
            # Throwaway output for the fused square+reduce: stride-0
            # broadcast of a [P,1] tile (qr.py's safe_norm trick).
            dummy = small.tile([P, 1], _DT, tag="dummy")

            for i, (t, c0, w, ring) in enumerate(CHUNKS):
                pool, tag = pools[w]
                at = pool.tile([P, w], _DT, tag=tag)
                eng = nc.sync if ring == 0 else nc.scalar
                eng.dma_start(out=at[:], in_=a_tiles[t][:, c0 : c0 + w])
                nc.scalar.activation(
                    out=dummy.broadcast_to(at.shape),
                    in_=at[:],
                    func=mybir.ActivationFunctionType.Square,
                    accum_out=racc[:, i : i + 1],
                )

            nc.sync.dma_start(out=out[:], in_=racc[:])

    nc.compile()
    return nc


_nc_cache = {}


def _get_nc():
    if "nc" not in _nc_cache:
        _nc_cache["nc"] = build_nc()
    return _nc_cache["nc"]


def _run(inputs, trace=False):
    X = np.ascontiguousarray(np.asarray(inputs["X"], dtype=np.float32))
    A = np.ascontiguousarray(np.asarray(inputs["A"], dtype=np.float32))

    nc = _get_nc()
    in_maps = [
        {"a_shard": A[c * ROWS : (c + 1) * ROWS]} for c in range(NCORES)
    ]
    res = run_bass_kernel_spmd(
        nc, in_maps, core_ids=list(range(NCORES)), trace=trace
    )

    # Host epilogue in float64: fold chunk partials per tile, weight by
    # x^2, add the diagonal term.
    X64 = X.astype(np.float64)
    total = 0.0
    for c in range(NCORES):
        r = res.results[c]["out"].astype(np.float64)  # [P, NCH]
        # tiles 0..5 are single chunks (cols 0..5); tiles 6/7 are the
        # even/odd taper columns 6,8,10,12 / 7,9,11,13.
        rt = np.empty((P, TILES), dtype=np.float64)
        rt[:, :6] = r[:, :6]
        rt[:, 6] = r[:, [6, 8, 10, 12]].sum(axis=1)
        rt[:, 7] = r[:, [7, 9, 11, 13]].sum(axis=1)
        # x for row-tile t, partition p is X[core*ROWS + t*128 + p]
        xc = X64[c * ROWS : (c + 1) * ROWS].reshape(TILES, P).T  # [P, T]
        total += (xc * xc * rt).sum()

    d64 = np.asarray(A.diagonal(), dtype=np.float64)
    total += -2.0 * float(X64 @ d64) + float(N)
    return np.float32(total), res


def kernel(**inputs):
    out, _ = _run(inputs, trace=False)
    return out
